# revision 45
# baseline (speedup 1.0000x reference)
"""Trainium2 Bass kernel for causal multi-head attention (B=4, T=2048, C=1024, H=16).

Sharding: 8 NeuronCores = batch (4) x head-group (2). Each core computes, for
its batch b and its 8 heads:
  - QKV projections with column-sharded weights (Q^T/K^T in [D*,T] layout,
    V in [T, D*] layout),
  - causal attention with an appended validity/row-sum column on V
    (flash-style unnormalized accumulation + fused denominator),
  - row-sharded output projection producing a partial [T, C] output (bf16).
The host sums the two head-group partials per batch and adds the output bias.

All matmul operands are bf16 (full PE rate, half the SBUF/DMA footprint of
f32r); PSUM accumulation stays fp32. The attention result y^T stays resident
in SBUF (bf16) and feeds the output projection directly -- no DRAM bounce.
Causal masking happens on the es (exp scores) tiles in SBUF *after* the exp,
off the PE->Act critical path. DMA instructions are batched with multi-dim
access patterns (the shared HWDGE descriptor generator costs ~625ns per DMA
instruction regardless of size).

Schedule: one rolling loop -- attention for query-block qb is emitted
interleaved with the projections of block qb+1. Each attention unit is split
into a score/exp phase and an av-drain phase; the drain of unit j is emitted
after the next projection chain so the Scalar-engine exp latency hides under
PE-bound projection matmuls. Projection chains rotate over two PSUM tags so
a chain's PSUM drain (DVE copy) overlaps the next chain.
"""

import numpy as np
from contextlib import ExitStack

B, T, C, H = 4, 2048, 1024, 16
D = C // H            # 64
CL = C // 2           # 512 local channels (8 heads) per core
NCI = C // 128        # 8 contraction tiles for projections
PAIR_BLK = 192        # v_sb columns per head pair: [V_e(64) | valid(1) | gap(63) | V_o(64)]

_CACHE = {}

# schedule-pipelining knobs
AV_LAG = 2
ST_BUFS = 3
ES_BUFS = 8


def _build(t_len):
    import concourse.bass as bass  # noqa: F401
    import concourse.tile as tile
    from concourse import bacc, mybir

    dt = mybir.dt
    AF = mybir.ActivationFunctionType
    Alu = mybir.AluOpType

    NT = t_len // 128     # t tiles
    NB = t_len // 512     # t blocks

    nc = bacc.Bacc("TRN2", target_bir_lowering=False, debug=False,
                   enable_asserts=False, num_devices=8)

    xt_d = nc.dram_tensor("xt", (C, t_len), dt.bfloat16, kind="ExternalInput").ap()
    wq_d = nc.dram_tensor("wq", (C, CL), dt.bfloat16, kind="ExternalInput").ap()
    wk_d = nc.dram_tensor("wk", (C, CL), dt.bfloat16, kind="ExternalInput").ap()
    wv_d = nc.dram_tensor("wv", (C, CL), dt.bfloat16, kind="ExternalInput").ap()
    wp_d = nc.dram_tensor("wp", (CL, C), dt.bfloat16, kind="ExternalInput").ap()
    aux_d = nc.dram_tensor("aux", (128, NT + 8), dt.float32, kind="ExternalInput").ap()
    tri_d = nc.dram_tensor("tri", (128, 128), dt.bfloat16, kind="ExternalInput").ap()
    out_d = nc.dram_tensor("out", (t_len, C), dt.bfloat16, kind="ExternalOutput").ap()

    with tile.TileContext(nc) as tc, ExitStack() as octx:
        persist = octx.enter_context(tc.tile_pool(name="persist", bufs=1))

        # Small persistent tensors (aux = [vm(NT) | bq(4) | bk(4)])
        aux = persist.tile([128, NT + 8], dt.float32, tag="aux")
        tri = persist.tile([128, 128], dt.bfloat16, tag="tri")
        vm16 = aux[:, 0:NT]
        # warm the Scalar engine's Exp table while the first DMAs run, so the
        # first real exp doesn't pay the ~1.3us table load
        scr = persist.tile([1, 1], dt.float32, tag="scr")
        nc.vector.memset(scr[:], 0.0)
        nc.scalar.activation(scr[:], scr[:], mybir.ActivationFunctionType.Exp)
        bq_sb = aux[:, NT:NT + 4]
        bk_sb = aux[:, NT + 4:NT + 8]

        # Persistent activations (Q^T is rolled per t-block; K^T/V/y^T persist)
        kt_ = [persist.tile([128, t_len], dt.bfloat16, tag=f"kt{j}", name=f"kt{j}") for j in range(4)]
        vsb = [persist.tile([128, 4 * PAIR_BLK], dt.bfloat16, tag=f"v{t}",
                             name=f"v{t}") for t in range(NT)]
        ytil = [persist.tile([128, t_len], dt.bfloat16, tag=f"yt{j}", name=f"yt{j}")
                for j in range(4)]

        # ------- merged loop: projections for t-block tb, then attention qb=tb -------
        with (
            tc.tile_pool(name="pm", bufs=1) as pm,
            tc.tile_pool(name="psm", bufs=1, space="PSUM") as psm,
        ):
            wq_sb = pm.tile([128, NCI * CL], dt.bfloat16, tag="wqs", name="wqs")
            wk_sb = pm.tile([128, NCI * CL], dt.bfloat16, tag="wks", name="wks")
            wv_sb = pm.tile([128, NCI * CL], dt.bfloat16, tag="wvs", name="wvs")
            wqr = wq_sb[:].rearrange("p (ci c) -> p ci c", ci=NCI)
            wkr = wk_sb[:].rearrange("p (ci c) -> p ci c", ci=NCI)
            wvr = wv_sb[:].rearrange("p (ci c) -> p ci c", ci=NCI)
            # batched loads with [p, ci, c] access patterns; wq/x split in
            # halves so the first Q chain can start after ~1MB instead of 2MB
            xs0 = pm.tile([128, NCI * 512], dt.bfloat16, tag="xs", name="xs0", bufs=2)
            xs0r = xs0[:].rearrange("p (ci t) -> p ci t", ci=NCI)
            xdr = xt_d[:, 0:512].rearrange("(ci p) t -> p ci t", p=128)
            wqd = wq_d[:].rearrange("(ci p) c -> p ci c", p=128)
            for lo, hi in ((0, 2), (2, 4), (4, 8)):
                nc.sync.dma_start(wqr[:, lo:hi, :], wqd[:, lo:hi, :])
                nc.sync.dma_start(xs0r[:, lo:hi, :], xdr[:, lo:hi, :])
            nc.sync.dma_start(aux[:], aux_d[:])
            nc.sync.dma_start(tri[:], tri_d[:])
            nc.sync.dma_start(
                wkr, wk_d[:].rearrange("(ci p) c -> p ci c", p=128))
            nc.sync.dma_start(
                wvr, wv_d[:].rearrange("(ci p) c -> p ci c", p=128))
            xs0 = xs0r
            zero4 = pm.tile([128, 4], dt.float32, tag="zero4", name="zero4")
            nc.vector.memset(zero4[:], 0.0)
            on128 = pm.tile([128, 1], dt.float32, tag="on128", name="on128")
            nc.vector.memset(on128[:], 1.0)

            # projection PSUM rotation: hide each chain's drain under the next
            proj_ps = [("qk", 1), ("vps", 2)]
            proj_i = [0]

            def next_ps(width=512):
                tag, bufs = proj_ps[proj_i[0] % 2]
                proj_i[0] += 1
                return psm.tile([128, width], dt.float32, tag=tag, bufs=bufs,
                                name=f"ps_{tag}")

            def emit_unit_phase1(qb, j, qtrj):
                q0 = qb * 512
                # av PSUM tiles allocate lazily at the first av matmul so a
                # qb=0 unit (which defers every av) doesn't pin the tag while
                # the previous unit's drain is still pending
                avs = [None]
                n_kt = qb * 4 + 4

                def get_avs():
                    if avs[0] is None:
                        av0 = psm.tile([65, 512], dt.float32, tag="av0",
                                       name="av0")
                        av1 = psm.tile([128, 512], dt.float32, tag="av1",
                                       name="av1")
                        avs[0] = (av0, av1)
                    return avs[0]

                def emit_av(item):
                    h01, kt2, c02, width2, es2 = item
                    vofs = j * PAIR_BLK + h01 * 64
                    lw = 65 if h01 == 0 else 128
                    nc.tensor.matmul(
                        get_avs()[h01][:, c02:512],
                        vsb[kt2][:, vofs:vofs + lw],
                        es2[:, 0:width2],
                        start=(kt2 == 0), stop=(kt2 == n_kt - 1))

                # small units (qb=0) keep every av in the deferred drain; the
                # first exps would otherwise stall the in-order PE queue.
                # the last block's units keep extra full-width avs deferred so
                # the drain has PE cover for the final exps' latency
                if n_kt <= 4:
                    inline_thresh = 2 * n_kt
                elif qb == NB - 1:
                    inline_thresh = 12
                else:
                    inline_thresh = 2 * AV_LAG
                pend = []
                for kt in range(n_kt):
                    off = kt * 128 - q0
                    c0 = min(max(off, 0), 384)
                    width = 512 - c0
                    for h01 in range(2):
                        hb = h01 * 64
                        st = psm.tile([128, 512], dt.float32, tag="st",
                                      bufs=ST_BUFS)
                        nc.tensor.matmul(
                            st[:, 0:width],
                            kt_[j][hb:hb + 64, kt * 128:(kt + 1) * 128],
                            qtrj[hb:hb + 64, c0:512],
                            start=True, stop=True, tile_position=(hb, 0))
                        es = pm.tile([128, 512], dt.bfloat16, tag=f"es{h01}",
                                     bufs=ES_BUFS)
                        nc.scalar.activation(es[:, 0:width], st[:, 0:width],
                                             AF.Exp, scale=0.125)
                        if off >= 0:
                            # causal mask: zero the upper triangle of the
                            # diagonal 128-col band (post-exp, SBUF side)
                            nc.vector.tensor_tensor(
                                es[:, off - c0:off - c0 + 128],
                                es[:, off - c0:off - c0 + 128],
                                tri[:], Alu.mult)
                        pend.append((h01, kt, c0, width, es))
                        while len(pend) > inline_thresh:
                            emit_av(pend.pop(0))
                return (qb, j, pend, emit_av, get_avs)

            def emit_unit_phase2(state, tail=False):
                qb, j, pend, emit_av, get_avs = state
                q0 = qb * 512
                # drain head-e avs first so the dens_e chain (copy -> hop to
                # partition 0 -> recip -> broadcast; partition_broadcast /
                # reciprocal only work from partition 0 on HW) overlaps the
                # head-o avs still running on PE
                for item in pend:
                    if item[0] == 0:
                        emit_av(item)
                av0, av1 = get_avs()
                sr = pm.tile([128, 512], dt.float32, tag="sr", bufs=1)
                nc.vector.tensor_copy(sr[64:65, :], av0[64:65, :])
                ra = pm.tile([1, 512], dt.float32, tag="ra", bufs=2)
                if not tail:
                    # off the critical path: DMA hop row 64 -> row 0
                    nc.sync.dma_start(ra[0:1, :], sr[64:65, :])
                for item in pend:
                    if item[0] == 1:
                        emit_av(item)
                if tail:
                    # latency-critical: hop to partition 0 via a K=1 fp32
                    # matmul (~850ns PE) instead of the ~3us DMA round trip
                    dn = next_ps()
                    nc.tensor.matmul(dn[0:1, :], on128[64:65, 0:1],
                                     sr[64:65, :], start=True, stop=True)
                    nc.vector.tensor_copy(ra[0:1, :], dn[0:1, :])
                rra = pm.tile([1, 512], dt.float32, tag="rra", bufs=2)
                nc.vector.reciprocal_approx_fast(out=rra[0:1, :], in_=ra[0:1, :])
                bca = pm.tile([128, 512], dt.float32, tag="bca", bufs=1)
                nc.gpsimd.partition_broadcast(bca[:, :], rra[0:1, :], channels=128)
                nc.vector.tensor_mul(ytil[j][0:64, q0:q0 + 512],
                                     av0[0:64, :], bca[0:64, :])
                sr2 = pm.tile([1, 512], dt.float32, tag="sr2", bufs=2)
                nc.vector.tensor_copy(sr2[0:1, :], av1[0:1, :])
                rrb = pm.tile([1, 512], dt.float32, tag="rrb", bufs=2)
                nc.vector.reciprocal_approx_fast(out=rrb[0:1, :], in_=sr2[0:1, :])
                bcb = pm.tile([128, 512], dt.float32, tag="bcb", bufs=1)
                nc.gpsimd.partition_broadcast(bcb[:, :], rrb[0:1, :], channels=128)
                nc.vector.tensor_mul(ytil[j][64:128, q0:q0 + 512],
                                     av1[64:128, :], bcb[64:128, :])

            def emit_proj(tts):
                for tt in tts:
                    po = pm.tile([128, 1024], dt.bfloat16, tag="po", bufs=3)
                    split = tt == NT - 1  # overlap the last write with cb=1
                    for cb in range(2):
                        pj = next_ps()
                        for j in range(4):
                            nc.tensor.matmul(
                                pj[:], ytil[j][:, tt * 128:(tt + 1) * 128],
                                wpr[:, j, cb, :], start=(j == 0), stop=(j == 3))
                        nc.vector.tensor_copy(po[:, cb * 512:(cb + 1) * 512], pj[:])
                        if split:
                            nc.sync.dma_start(
                                out_d[tt * 128:(tt + 1) * 128,
                                      cb * 512:(cb + 1) * 512],
                                po[:, cb * 512:(cb + 1) * 512])
                    if not split:
                        nc.sync.dma_start(out_d[tt * 128:(tt + 1) * 128, :], po[:])

            open_units = []

            def drain_unit(tail=False):
                if open_units:
                    emit_unit_phase2(open_units.pop(0), tail=tail)

            wp_sb = None
            prev_qtr = None
            for tb in range(NB):
                ts = slice(tb * 512, (tb + 1) * 512)
                if tb == 0:
                    xs = xs0
                else:
                    xst = pm.tile([128, NCI * 512], dt.bfloat16, tag="xs", bufs=2)
                    nc.sync.dma_start(
                        xst[:].rearrange("p (ci t) -> p ci t", ci=NCI),
                        xt_d[:, ts].rearrange("(ci p) t -> p ci t", p=128))
                    xs = xst[:].rearrange("p (ci t) -> p ci t", ci=NCI)
                # Q^T (rolling, this block only) and K^T (persistent)
                qtr = []
                for j in range(4):
                    ps = next_ps()
                    for ci in range(NCI):
                        nc.tensor.matmul(
                            ps[:], wqr[:, ci, j * 128:(j + 1) * 128], xs[:, ci, :],
                            start=(ci == 0), stop=(ci == NCI - 1))
                    qj = pm.tile([128, 512], dt.bfloat16, tag=f"qtr{j}", name=f"qtr{j}", bufs=2)
                    nc.vector.tensor_scalar_add(qj[:], ps[:], bq_sb[:, j:j + 1])
                    qtr.append(qj)
                    if tb == 1:
                        # qb=0 units are tiny (8 exps ~5us on Act vs 1.7us of
                        # PE per chain): keep two units open so each drain's
                        # exps get two chains + one score phase of cover
                        open_units.append(emit_unit_phase1(0, j, prev_qtr[j]))
                        if len(open_units) > 1:
                            drain_unit()
                    else:
                        drain_unit()
                        if prev_qtr is not None:
                            open_units.append(
                                emit_unit_phase1(tb - 1, j, prev_qtr[j]))
                if tb == NB - 1:
                    wp_sb = pm.tile([128, 8 * 512], dt.bfloat16, tag="wps",
                                    name="wps")
                    wpr = wp_sb[:].rearrange("p (j h c) -> p j h c", j=4, h=2)
                    nc.sync.dma_start(
                        wpr, wp_d[:].rearrange("(j p) (h c) -> p j h c",
                                               p=128, h=2))
                for j in range(4):
                    ps = next_ps()
                    for ci in range(NCI):
                        nc.tensor.matmul(
                            ps[:], wkr[:, ci, j * 128:(j + 1) * 128], xs[:, ci, :],
                            start=(ci == 0), stop=(ci == NCI - 1))
                    nc.vector.tensor_scalar_add(kt_[j][:, ts], ps[:], bk_sb[:, j:j + 1])
                    if j == 0:
                        drain_unit()
                # V tiles for this block
                for tt in range(tb * 4, tb * 4 + 4):
                    lt = tt % 4
                    ps = next_ps(CL)
                    for ci in range(NCI):
                        nc.tensor.matmul(
                            ps[:], xs[:, ci, lt * 128:(lt + 1) * 128], wvr[:, ci, :],
                            start=(ci == 0), stop=(ci == NCI - 1))
                    vt = vsb[tt]
                    vmc = vm16[:, tt:tt + 1]
                    ve_out = vt[:].rearrange("p (q b) -> p q b", b=PAIR_BLK)[:, :, 0:64]
                    ve_in = ps[:].rearrange("p (q b) -> p q b", b=128)[:, :, 0:64]
                    nc.vector.tensor_scalar_mul(ve_out, ve_in, vmc)
                    vo_out = vt[:].rearrange("p (q b) -> p q b", b=PAIR_BLK)[:, :, 128:192]
                    vo_in = ps[:].rearrange("p (q b) -> p q b", b=128)[:, :, 64:128]
                    nc.vector.tensor_scalar_mul(vo_out, vo_in, vmc)
                    vv_out = vt[:].rearrange("p (q b) -> p q b", b=PAIR_BLK)[:, :, 64:65]
                    nc.vector.tensor_scalar_add(vv_out, zero4[:], vmc)
                    vg_out = vt[:].rearrange("p (q b) -> p q b", b=PAIR_BLK)[:, :, 65:128]
                    vg_in = ps[:].rearrange("p (q b) -> p q b", b=128)[:, :, 65:128]
                    nc.vector.tensor_scalar_mul(vg_out, vg_in, vmc)
                if tb == NB - 1 and NB > 1:
                    emit_proj(range(0, 3))
                prev_qtr = qtr

            # ---- tail: last-block attention interleaved with the projection ----
            # proj for blocks qb <= NB-2 interleaves with the tail units;
            # the last block's tiles go after its final unit
            done = (NB - 1) * 4  # y^T rows complete pre-tail (0..3 emitted in-loop)
            base = 3 if NB > 1 else 0
            # keep two norm-independent tts (from completed blocks) in reserve
            # after the last unit's drain so its norm latency hides under them
            resv = max(base, done - 3)
            for j in range(4):
                drain_unit(tail=True)
                open_units.append(emit_unit_phase1(NB - 1, j, prev_qtr[j]))
                if j < 3 and resv > base:
                    lo = base + j * (resv - base) // 3
                    hi = base + (j + 1) * (resv - base) // 3
                    emit_proj(range(lo, hi))
            drain_unit(tail=True)
            emit_proj(range(resv if NB > 1 else 0, NT))

    nc.compile()
    return nc


def _shard_inputs(x, attention_mask, Wq, bq, Wk, bk, Wv, bv, Wp, t_len):
    import ml_dtypes
    bf16 = ml_dtypes.bfloat16
    NT = t_len // 128
    r_, c_ = np.arange(128)[:, None], np.arange(128)[None, :]
    tri = (c_ >= r_).astype(bf16)  # causal keep-mask for the diagonal band
    in_maps = []
    for core in range(8):
        b, hg = core // 2, core % 2
        hs = slice(hg * CL, (hg + 1) * CL)
        aux = np.empty((128, NT + 8), np.float32)
        aux[:, 0:NT] = (attention_mask[b, :t_len].astype(np.float32)
                        .reshape(NT, 128).T)
        aux[:, NT:NT + 4] = np.asarray(bq[hs], np.float32).reshape(4, 128).T
        aux[:, NT + 4:NT + 8] = np.asarray(bk[hs], np.float32).reshape(4, 128).T
        in_maps.append({
            "xt": np.ascontiguousarray(x[b, :t_len].T).astype(bf16),
            "wq": np.ascontiguousarray(Wq[:, hs]).astype(bf16),
            "wk": np.ascontiguousarray(Wk[:, hs]).astype(bf16),
            "wv": np.ascontiguousarray(Wv[:, hs]).astype(bf16),
            "wp": np.ascontiguousarray(Wp[hs, :]).astype(bf16),
            "aux": aux,
            "tri": tri,
        })
    return in_maps


def kernel(**inputs):
    from concourse import bass_utils

    t_len = T
    key = ("nc", t_len)
    if key not in _CACHE:
        _CACHE[key] = _build(t_len)
    nc = _CACHE[key]

    x = np.asarray(inputs["x"], dtype=np.float32)
    am = np.asarray(inputs["attention_mask"])
    in_maps = _shard_inputs(
        x, am, np.asarray(inputs["Wq"], np.float32), np.asarray(inputs["bq"], np.float32),
        np.asarray(inputs["Wk"], np.float32), np.asarray(inputs["bk"], np.float32),
        np.asarray(inputs["Wv"], np.float32), np.asarray(inputs["bv"], np.float32),
        np.asarray(inputs["Wp"], np.float32), t_len)

    res = bass_utils.run_bass_kernel_spmd(nc, in_maps, core_ids=list(range(8)))
    # attention rows sum to 1, so the V bias contributes bv @ Wp, a constant
    # row folded into the output bias on the host
    bias = (np.asarray(inputs["bp"], np.float32)
            + np.asarray(inputs["bv"], np.float32)
            @ np.asarray(inputs["Wp"], np.float32))
    out = np.empty((B, T, C), dtype=np.float32)
    for b in range(B):
        out[b] = (res.results[2 * b]["out"].astype(np.float32)
                  + res.results[2 * b + 1]["out"].astype(np.float32) + bias)
    return out
